# revision 6
# baseline (speedup 1.0000x reference)
"""GCN classifier (512 batched graphs x 200 nodes x 6400 edges) on 8 Trainium2 cores.

Strategy (data/graph parallel per the sharding hint): 64 graphs per core.
The host ships each graph's dense 200x200 adjacency-count matrix A^T
(integer edge counts from a bincount of the edge list -- pure structural
preprocessing, exact in bf16) plus integer in/out degree tables.  On device,
all floating-point model math runs as dense streaming matmuls:

  t1 = A s           (scalar message passing, N=1 column matmuls)
  u  = relu(a2 (x) W1 + invout (x) b1)   (rank-1 K=1 matmuls + Act relu)
  aggT = u^T A^T     (two N=256 streaming matmuls per graph)
  h2T = W2^T (aggT * invin_bc); relu+bias+mean fused in one Act op
         (per-partition bias + accum_out readout)
  MLP head + softmax as small dense matmuls.

No collectives: outputs are row-wise per graph; host concatenates the
8 per-core [64, 10] blocks.
"""

import sys

sys.path.insert(0, "/opt/trn_rl_repo")

import numpy as np
import ml_dtypes

from concourse import bacc, bass, mybir, tile
from concourse.bass_utils import run_bass_kernel_spmd
from concourse.masks import make_identity

# Problem constants (hardcoded per the task contract).
N_GRAPHS = 512
NODES_PER_G = 200
EDGES_PER_G = 6400
E = N_GRAPHS * EDGES_PER_G
HID = 128
NCLS = 10
N_CORES = 8
GPC = N_GRAPHS // N_CORES          # graphs per core = 64
EPC = GPC * EDGES_PER_G            # edges per core
NPAD = 256                         # padded nodes per graph (2 tiles of 128)

F32 = mybir.dt.float32
BF16 = mybir.dt.bfloat16
FP8 = mybir.dt.float8e4
AF = mybir.ActivationFunctionType
ALU = mybir.AluOpType

MLP_DIMS = [(HID, 512), (512, 1024), (1024, 1024), (1024, 512), (512, NCLS)]

_PROGRAM = None
LAST_RESULTS = None  # BassKernelResults of the most recent run (for test.py)


# --------------------------------------------------------------------------
# Host preprocessing: integer structural data (bincounts of the edge list)
# --------------------------------------------------------------------------

def _preprocess(src, dst):
    """Per-core dense adjacency counts + degree tables.

    at[c]:  [128, 128, 256] bf16, at[s, g*2+st, d] = #edges (st*128+s -> d)
            in graph g of core c (exact small-integer counts).
    deg[c]: [64, 512] f32, cols 0:256 in-degree, 256:512 out-degree.
    """
    src = np.asarray(src).astype(np.int64)
    dst = np.asarray(dst).astype(np.int64)
    g = np.arange(E, dtype=np.int64) // EDGES_PER_G
    sl = src - g * NODES_PER_G
    dl = dst - g * NODES_PER_G
    at_list, deg_list = [], []
    for c in range(N_CORES):
        lo, hi = c * EPC, (c + 1) * EPC
        gl = g[lo:hi] - c * GPC
        slc, dlc = sl[lo:hi], dl[lo:hi]
        key = (gl * 2 + (slc >> 7)) * (128 * 256) + (slc & 127) * 256 + dlc
        cnt = np.bincount(key, minlength=GPC * 2 * 128 * 256)
        assert cnt.max() <= 16, "edge multiplicity exceeds fp8e4 range"
        at = (cnt.reshape(GPC, 2, 128, 256).transpose(2, 0, 1, 3)
              .reshape(128, 2 * GPC, 256))
        at_list.append(at.astype(mybir.dt.np(FP8)))
        ind = np.bincount(gl * 256 + dlc, minlength=GPC * 256)
        outd = np.bincount(gl * 256 + slc, minlength=GPC * 256)
        deg = np.concatenate([ind.reshape(GPC, 256), outd.reshape(GPC, 256)],
                             axis=1)
        deg_list.append(deg.astype(np.float32))
    return at_list, deg_list


def _prep_weights(W1, b1, W2, b2, Wa, ba, Wb, bb, Wc, bc, Wd, bd, We, be):
    bf = ml_dtypes.bfloat16
    base = {
        "w1r": np.asarray(W1, np.float32).reshape(1, HID).astype(bf),
        "b1r": np.asarray(b1, np.float32).reshape(1, HID).astype(bf),
        "w2": np.ascontiguousarray(np.asarray(W2, np.float32)).astype(bf),
        "b2c": np.ascontiguousarray(
            np.asarray(b2, np.float32).reshape(HID, 1)),
    }
    for li, (w, bvec) in enumerate(
            zip((Wa, Wb, Wc, Wd, We), (ba, bb, bc, bd, be))):
        w = np.asarray(w, np.float32)
        bvec = np.asarray(bvec, np.float32)
        fi, fo = w.shape
        base[f"mw{li}"] = np.ascontiguousarray(
            w.reshape(fi // 128, 128, fo).transpose(1, 0, 2)).astype(bf)
        if fo >= 128:
            bcol = np.ascontiguousarray(bvec.reshape(-1, 128).T)
        else:
            bcol = np.zeros((128, 1), np.float32)
            bcol[:fo, 0] = bvec
        base[f"mbc{li}"] = bcol
    return base


# --------------------------------------------------------------------------
# Bass program (fixed shape -- no data-dependent structure)
# --------------------------------------------------------------------------

def _build_program():
    nc = bacc.Bacc(None, target_bir_lowering=False, debug=False)

    NCHUNK = 8                       # gst DMA chunks
    CW = 2 * GPC // NCHUNK           # gst per chunk = 16

    at_d = nc.dram_tensor("at", [128, 2 * GPC, 256], FP8, kind="ExternalInput")
    deg_d = nc.dram_tensor("deg", [GPC, 512], F32, kind="ExternalInput")
    w1r_d = nc.dram_tensor("w1r", [1, HID], BF16, kind="ExternalInput")
    b1r_d = nc.dram_tensor("b1r", [1, HID], BF16, kind="ExternalInput")
    w2_d = nc.dram_tensor("w2", [HID, HID], BF16, kind="ExternalInput")
    b2c_d = nc.dram_tensor("b2c", [HID, 1], F32, kind="ExternalInput")
    mw_d, mbc_d = [], []
    for li, (fi, fo) in enumerate(MLP_DIMS):
        mw_d.append(nc.dram_tensor(f"mw{li}", [128, fi // 128, fo], BF16,
                                   kind="ExternalInput"))
        mbc_d.append(nc.dram_tensor(f"mbc{li}", [128, max(1, fo // 128)], F32,
                                    kind="ExternalInput"))
    out_d = nc.dram_tensor("out", [GPC, NCLS], F32, kind="ExternalOutput")

    with tile.TileContext(nc) as tc:
        with (
            tc.tile_pool(name="glob", bufs=1) as gp,
            tc.tile_pool(name="uh2", bufs=2, space="PSUM") as upsp,
            tc.tile_pool(name="aggps", bufs=2, space="PSUM") as aggp,
            tc.tile_pool(name="bcsb", bufs=3) as bcp,
            tc.tile_pool(name="mps", bufs=2, space="PSUM") as mpsp,
            tc.tile_pool(name="usb", bufs=3) as usbp,
            tc.tile_pool(name="agisb", bufs=3) as agip,
            tc.tile_pool(name="h2scr", bufs=2) as h2sp,
        ):
            # ---------------- input DMAs (SP queue, streaming order) -------
            deg = gp.tile([GPC, 512], F32)
            nc.sync.dma_start(deg[:], deg_d[:])
            wb = gp.tile([1, 2 * HID], BF16)
            w1r = wb[0:1, 0:HID]
            b1r = wb[0:1, HID:2 * HID]
            nc.sync.dma_start(w1r, w1r_d[:])
            nc.sync.dma_start(b1r, b1r_d[:])
            w2 = gp.tile([HID, HID], BF16)
            nc.sync.dma_start(w2[:], w2_d[:])
            b2c = gp.tile([HID, 1], F32)
            nc.sync.dma_start(b2c[:], b2c_d[:])
            at8 = []
            for k in range(NCHUNK):
                t = gp.tile([128, CW, 256], FP8, name=f"at{k}")
                nc.sync.dma_start(t[:], at_d[:, k * CW:(k + 1) * CW, :])
                at8.append(t)
            mw, mbc = [], []
            for li, (fi, fo) in enumerate(MLP_DIMS):
                w = gp.tile([128, fi // 128, fo], BF16, name=f"mw{li}")
                nc.sync.dma_start(w[:], mw_d[li][:])
                mw.append(w)
                b = gp.tile([128, max(1, fo // 128)], F32, name=f"mbc{li}")
                nc.sync.dma_start(b[:], mbc_d[li][:])
                mbc.append(b)

            def at_sl(gst):
                return at8[gst // CW][:, gst % CW, :]

            # ---------------- constants ----------------
            ident = gp.tile([128, 128], F32)
            make_identity(nc, ident[:])

            # ---------------- phase A: degrees -> norms (vectorized) -------
            inv = gp.tile([GPC, 512], F32)
            nc.vector.tensor_scalar(out=inv[:], in0=deg[:], scalar1=1.0,
                                    scalar2=None, op0=ALU.max)
            nc.scalar.sqrt(inv[:], inv[:])
            nc.vector.reciprocal(inv[:], inv[:])
            s_G = gp.tile([GPC, 256], F32)
            nc.vector.tensor_tensor(out=s_G[:], in0=deg[:, 0:256],
                                    in1=inv[:, 256:512], op=ALU.mult)
            invio = gp.tile([GPC, 256], F32)
            nc.vector.tensor_tensor(out=invio[:], in0=inv[:, 0:256],
                                    in1=inv[:, 256:512], op=ALU.mult)
            invb = gp.tile([GPC, 512], BF16)
            nc.vector.tensor_copy(invb[:], inv[:])
            # P0 staging (single-partition row layouts) via SBUF->SBUF DMA.
            # All at partition 0: matmul operands starting at partition 32/64
            # crash the runtime (probed), so each vector gets its own column
            # range of one partition-0 tile.
            NP0 = GPC * 256
            p0 = gp.tile([1, 3 * NP0], BF16)
            innb0 = p0[0:1, 0:NP0]
            outb0 = p0[0:1, NP0:2 * NP0]
            a20 = p0[0:1, 2 * NP0:3 * NP0]
            nc.gpsimd.dma_start(innb0, invb[:, 0:256])
            nc.gpsimd.dma_start(outb0, invb[:, 256:512])
            # s columns: transpose s_G -> s_NT [128 node, st*64+g] bf16
            st_ps = mpsp.tile([128, 128], F32, tag="m", name="st_ps")
            for st in range(2):
                nc.tensor.transpose(st_ps[:, st * 64:(st + 1) * 64],
                                    s_G[:, st * 128:(st + 1) * 128],
                                    ident[0:GPC, 0:GPC])
            s_NT = gp.tile([128, 128], BF16)
            nc.vector.tensor_copy(s_NT[:], st_ps[:])

            # ---------------- phase B: t1 = A s (column matmuls) -----------
            t1_ps = mpsp.tile([128, 128], F32, tag="m", name="t1_ps")
            for g in range(GPC):
                for dh in range(2):
                    col = dh * 64 + g
                    for st in range(2):
                        nc.tensor.matmul(
                            t1_ps[:, col:col + 1],
                            lhsT=at_sl(2 * g + st)[:, dh * 128:(dh + 1) * 128],
                            rhs=s_NT[:, st * 64 + g:st * 64 + g + 1],
                            start=(st == 0), stop=(st == 1),
                            skip_group_check=True)
            t1sb = gp.tile([128, 128], F32)
            nc.vector.tensor_copy(t1sb[:], t1_ps[:])
            t1g_ps = mpsp.tile([GPC, 256], F32, tag="m", name="t1g_ps")
            for dh in range(2):
                nc.tensor.transpose(t1g_ps[:, dh * 128:(dh + 1) * 128],
                                    t1sb[:, dh * 64:(dh + 1) * 64],
                                    ident[:])
            a2_G = gp.tile([GPC, 256], BF16)
            nc.vector.tensor_tensor(out=a2_G[:], in0=t1g_ps[:], in1=invio[:],
                                    op=ALU.mult)
            nc.gpsimd.dma_start(a20, a2_G[:])

            # ---------------- phase C: per-graph pipeline ------------------
            hgacc = gp.tile([128, GPC], F32)
            u_sb, agi_sb, agg_ps, bc_ps, h2_ps = {}, {}, {}, {}, {}

            def stage1(g):  # u build + relu, invin broadcast
                ups = upsp.tile([128, 256], F32, tag="u", name="u_ps")
                for st in range(2):
                    o = g * 256 + st * 128
                    nc.tensor.matmul(
                        ups[:, st * 128:(st + 1) * 128],
                        lhsT=a20[:, o:o + 128], rhs=w1r,
                        start=True, stop=False, skip_group_check=True)
                    nc.tensor.matmul(
                        ups[:, st * 128:(st + 1) * 128],
                        lhsT=outb0[:, o:o + 128], rhs=b1r,
                        start=False, stop=True, skip_group_check=True)
                bc = bcp.tile([128, 256], BF16, tag="bc", name="bc_sb")
                nc.gpsimd.partition_broadcast(
                    bc[:], innb0[:, g * 256:(g + 1) * 256])
                bc_ps[g] = bc
                usb = usbp.tile([128, 256], BF16, tag="usb", name="u_sb")
                nc.scalar.activation(usb[:], ups[:], AF.Relu)
                u_sb[g] = usb

            def stage2(g):  # aggregation + invin scale
                agg = aggp.tile([128, 256], F32, tag="agg", name="agg_ps")
                for st in range(2):
                    nc.tensor.matmul(
                        agg[:], lhsT=u_sb[g][:, st * 128:(st + 1) * 128],
                        rhs=at_sl(2 * g + st)[:],
                        start=(st == 0), stop=(st == 1))
                agi = agip.tile([128, 256], BF16, tag="agi", name="agi_sb")
                nc.vector.tensor_tensor(out=agi[:], in0=agg[:],
                                        in1=bc_ps[g][:], op=ALU.mult)
                agi_sb[g] = agi
                del u_sb[g], bc_ps[g]

            def stage3(g):  # W2 + fused bias/relu/readout
                h2 = upsp.tile([128, 256], F32, tag="u", name="h2_ps")
                nc.tensor.matmul(h2[:], lhsT=w2[:], rhs=agi_sb[g][:],
                                 start=True, stop=True)
                scr = h2sp.tile([128, NODES_PER_G], BF16, tag="scr",
                                name="h2scr")
                nc.scalar.activation(scr[:], h2[:, 0:NODES_PER_G], AF.Relu,
                                     bias=b2c[:], scale=1.0,
                                     accum_out=hgacc[:, g:g + 1])
                del agi_sb[g]

            for i in range(GPC + 2):
                if i < GPC:
                    stage1(i)
                if 1 <= i <= GPC:
                    stage2(i - 1)
                if 2 <= i:
                    stage3(i - 2)

            # ---------------- phase D: MLP head + softmax ------------------
            hgbf = gp.tile([128, 1, GPC], BF16)
            nc.vector.tensor_scalar(out=hgbf[:, 0, :], in0=hgacc[:],
                                    scalar1=1.0 / NODES_PER_G, scalar2=None,
                                    op0=ALU.mult)
            x = hgbf
            x5 = gp.tile([NCLS, GPC], F32)
            for li, (fi, fo) in enumerate(MLP_DIMS):
                itiles = fi // 128
                otiles = max(1, fo // 128)
                m = 128 if fo >= 128 else fo
                xn = gp.tile([128, otiles, GPC], BF16, name=f"x{li}")
                for ot in range(otiles):
                    ps = mpsp.tile([128, 128], F32, tag="m", name="mlp_ps")
                    for it in range(itiles):
                        nc.tensor.matmul(
                            ps[0:m, 0:GPC],
                            lhsT=mw[li][:, it, ot * 128:ot * 128 + m],
                            rhs=x[:, it, :], start=(it == 0),
                            stop=(it == itiles - 1))
                    if li < len(MLP_DIMS) - 1:
                        nc.scalar.activation(xn[:, ot, :], ps[:, 0:GPC],
                                             AF.Relu,
                                             bias=mbc[li][:, ot:ot + 1])
                    else:
                        nc.scalar.activation(x5[:], ps[0:NCLS, 0:GPC],
                                             AF.Identity,
                                             bias=mbc[li][0:NCLS, 0:1])
                x = xn

            tr_ps = mpsp.tile([128, 128], F32, tag="m", name="tr_ps")
            nc.tensor.transpose(tr_ps[0:GPC, 0:NCLS], x5[:],
                                ident[0:NCLS, 0:NCLS])
            sm = gp.tile([GPC, NCLS], F32)
            nc.vector.tensor_copy(sm[:], tr_ps[0:GPC, 0:NCLS])
            mx = gp.tile([GPC, 1], F32)
            nc.vector.tensor_reduce(out=mx[:], in_=sm[:],
                                    axis=mybir.AxisListType.X, op=ALU.max)
            nc.vector.tensor_scalar(out=sm[:], in0=sm[:], scalar1=mx[:],
                                    scalar2=None, op0=ALU.subtract)
            nc.scalar.activation(sm[:], sm[:], AF.Exp)
            ssum = gp.tile([GPC, 1], F32)
            nc.vector.tensor_reduce(out=ssum[:], in_=sm[:],
                                    axis=mybir.AxisListType.X, op=ALU.add)
            rsum = gp.tile([GPC, 1], F32)
            nc.vector.reciprocal(rsum[:], ssum[:])
            probs = gp.tile([GPC, NCLS], F32)
            nc.vector.tensor_scalar(out=probs[:], in0=sm[:], scalar1=rsum[:],
                                    scalar2=None, op0=ALU.mult)
            nc.sync.dma_start(out_d[:], probs[:])

    nc.compile()
    return nc


# --------------------------------------------------------------------------
# Entry point
# --------------------------------------------------------------------------

def kernel(src, dst, W1, b1, W2, b2, Wa, ba, Wb, bb, Wc, bc, Wd, bd, We, be):
    global LAST_RESULTS, _PROGRAM
    at_list, deg_list = _preprocess(src, dst)
    if _PROGRAM is None:
        _PROGRAM = _build_program()
    nc = _PROGRAM
    base = _prep_weights(W1, b1, W2, b2, Wa, ba, Wb, bb, Wc, bc, Wd, bd,
                         We, be)
    in_maps = [dict(base, at=at_list[c], deg=deg_list[c])
               for c in range(N_CORES)]
    LAST_RESULTS = run_bass_kernel_spmd(nc, in_maps, list(range(N_CORES)))
    out = np.concatenate(
        [LAST_RESULTS.results[c]["out"] for c in range(N_CORES)], axis=0)
    return out.astype(np.float32)


def measure_exec_ns(nc, in_map, iters=32, warmup=4):
    """Marginal per-execution device time of one core's program."""
    import time as _time
    import jax
    from concourse import bass2jax, mybir as _mb

    bass2jax.install_neuronx_cc_hook()
    partition_name = (nc.partition_id_tensor.name
                      if nc.partition_id_tensor else None)
    in_names, out_names, out_avals, zero_outs = [], [], [], []
    for alloc in nc.m.functions[0].allocations:
        if not isinstance(alloc, _mb.MemoryLocationSet):
            continue
        name = alloc.memorylocations[0].name
        if alloc.kind == "ExternalInput":
            if name != partition_name:
                in_names.append(name)
        elif alloc.kind == "ExternalOutput":
            shape = tuple(alloc.tensor_shape)
            dtype = _mb.dt.np(alloc.dtype)
            out_names.append(name)
            out_avals.append(jax.core.ShapedArray(shape, dtype))
            zero_outs.append(np.zeros(shape, dtype))
    n_params = len(in_names)
    all_in_names = list(in_names) + list(out_names)
    if partition_name is not None:
        all_in_names.append(partition_name)

    def _make_body(k):
        def _body(*args):
            outs = None
            for _ in range(k):
                operands = list(args)
                if partition_name is not None:
                    operands.append(bass2jax.partition_id_tensor())
                outs = tuple(bass2jax._bass_exec_p.bind(
                    *operands, out_avals=tuple(out_avals),
                    in_names=tuple(all_in_names), out_names=tuple(out_names),
                    lowering_input_output_aliases=(),
                    sim_require_finite=True, sim_require_nnan=True, nc=nc))
            return outs
        return jax.jit(_body, keep_unused=True)

    lo = max(1, iters // 4)
    fnl = _make_body(lo)
    fnk = _make_body(iters)
    dev = jax.devices()[0]
    dev_in = [jax.device_put(np.asarray(in_map[n]), dev) for n in in_names]
    dev_zo = [jax.device_put(z, dev) for z in zero_outs]

    for _ in range(warmup):
        jax.block_until_ready(fnl(*dev_in, *dev_zo))
    tl = min(_timeit(lambda: jax.block_until_ready(fnl(*dev_in, *dev_zo)))
             for _ in range(4))
    jax.block_until_ready(fnk(*dev_in, *dev_zo))
    tk = min(_timeit(lambda: jax.block_until_ready(fnk(*dev_in, *dev_zo)))
             for _ in range(4))
    marginal = (tk - tl) / (iters - lo)
    return marginal * 1e9, tk / iters * 1e9


def _timeit(f):
    import time as _time
    t0 = _time.perf_counter()
    f()
    return _time.perf_counter() - t0
